# revision 1
# baseline (speedup 1.0000x reference)
"""Trainium2 Bass kernel for nn_ContextGatingSigmoidClassifier.

Math (eval mode):
  f_so = lrelu(W_so @ bn_so(x_so) + b_so)        x: [B,2048,N,H,W]
  f_c  = lrelu(W_c  @ bn_c(x_c)  + b_c)
  f    = concat -> bn1 -> W1 -> bn2 -> lrelu -> W2 -> mean(H,W) -> sigmoid > 0.5

All BatchNorms are eval-mode affine maps, so they fold into the adjacent
linear layers (done host-side in fp64):
  A_so = W_so * s_so ; a_so = W_so @ t_so + b_so          (s,t: bn scale/shift)
  A1   = diag(s2) W1 diag(s1) ; a1 = s2*(W1 @ t1 + b1) + t2
Final threshold: sigmoid(mean) > 0.5  <=>  sum_hw(W2 @ h) > -49*b2.

Device mapping: data-parallel over batch (4 per core, 8 cores). Weights
replicated. Per core, per batch element b:
  x[b] is [2048, 588] (channels x positions). Matmuls keep channels on
  SBUF partitions (K-chunks of 128), positions on the free dim (2 tiles
  of 294 = one PSUM bank each). All matmul operands fp16 (PSUM accum is
  fp32); fp32->fp16 happens inside the input DMA (SWDGE cast).
  Channel->partition mapping for x is interleaved (partition p holds
  channels 16p..16p+15) so each DMA descriptor reads 37.6KB contiguous;
  layer-1 weights are permuted host-side to match.
"""

import numpy as np

import concourse.bass as bass  # noqa: F401  (engine types referenced via nc)
import concourse.tile as tile
from concourse import bacc, mybir
from concourse.bass_utils import run_bass_kernel_spmd

F16 = mybir.dt.float16
F32 = mybir.dt.float32

B, C, NN, HW = 32, 2048, 12, 49
NHW = NN * HW            # 588
N_CORES = 8
BPC = B // N_CORES       # 4 batch elements per core
MT = NHW // 2            # 294 columns = one PSUM bank of fp32
KC1 = C // 128           # 16 K-chunks, layer 1
OC1 = 512 // 128         # 4 output chunks, layer 1 (per branch)
KC2 = 1024 // 128        # 8 K-chunks, layer 2
OC2 = 256 // 128         # 2 output chunks, layer 2
EPS = 1e-5
SLOPE = 0.2


def _fold_params(d):
    """Fold BNs into linears, in fp64. Returns device-layout arrays."""
    g = {k: np.asarray(v, dtype=np.float64) for k, v in d.items()}

    def bn_st(p):
        s = g[f"{p}_g"] / np.sqrt(g[f"{p}_v"] + EPS)
        t = g[f"{p}_b"] - g[f"{p}_m"] * s
        return s, t

    s_so, t_so = bn_st("bn_so")
    s_c, t_c = bn_st("bn_c")
    s1, t1 = bn_st("bn1")
    s2, t2 = bn_st("bn2")

    A_so = g["W_so"] * s_so[None, :]                 # [512, 2048]
    a_so = g["W_so"] @ t_so + g["b_so"]              # [512]
    A_c = g["W_c"] * s_c[None, :]
    a_c = g["W_c"] @ t_c + g["b_c"]
    A1 = s2[:, None] * (g["W1"] * s1[None, :])       # [256, 1024]
    a1 = s2 * (g["W1"] @ t1 + g["b1"]) + t2          # [256]

    # layer-1 weights: chunk j holds channel 16p+j at partition p (matches
    # the contiguous-run x DMA layout). Stored p-major [128, k*m] so each
    # DMA descriptor is one 16KB contiguous per-partition run.
    def l1_prep(A):  # [512, 2048] -> [128, 16*512] fp16
        AT = A.T.reshape(128, 16, 512)               # [p, j, m] with ch = 16p+j
        return np.ascontiguousarray(AT.reshape(128, KC1 * 512)).astype(np.float16)

    wso = l1_prep(A_so)
    wc = l1_prep(A_c)
    w1 = np.ascontiguousarray(
        A1.T.reshape(KC2, 128, 256).transpose(1, 0, 2).reshape(128, KC2 * 256)
    ).astype(np.float16)
    w2 = np.ascontiguousarray(g["W2"].reshape(OC2, 128).T).astype(np.float16)  # [128, 2]
    bso = np.ascontiguousarray(a_so.reshape(OC1, 128).T).astype(np.float32)    # [128, 4]
    bc = np.ascontiguousarray(a_c.reshape(OC1, 128).T).astype(np.float32)
    b1 = np.ascontiguousarray(a1.reshape(OC2, 128).T).astype(np.float32)       # [128, 2]
    thresh = float(-HW * g["b2"][0])
    return wso, wc, w1, w2, bso, bc, b1, thresh


def build_bass(thresh, repeat=1, loop=1):
    """repeat: unrolled body repetitions; loop: on-device For_i wrapper
    around the whole body (used only for timing — reruns identical work)."""
    nc = bacc.Bacc("TRN2", target_bir_lowering=False, debug=False)

    xso_d = nc.dram_tensor("x_so", [BPC, C, NHW], F32, kind="ExternalInput").ap()
    xc_d = nc.dram_tensor("x_c", [BPC, C, NHW], F32, kind="ExternalInput").ap()
    wso_d = nc.dram_tensor("wso", [128, KC1 * 512], F16, kind="ExternalInput").ap()
    wc_d = nc.dram_tensor("wc", [128, KC1 * 512], F16, kind="ExternalInput").ap()
    w1_d = nc.dram_tensor("w1", [128, KC2 * 256], F16, kind="ExternalInput").ap()
    w2_d = nc.dram_tensor("w2", [128, OC2], F16, kind="ExternalInput").ap()
    bso_d = nc.dram_tensor("bso", [128, OC1], F32, kind="ExternalInput").ap()
    bc_d = nc.dram_tensor("bc", [128, OC1], F32, kind="ExternalInput").ap()
    b1_d = nc.dram_tensor("b1", [128, OC2], F32, kind="ExternalInput").ap()
    out_d = nc.dram_tensor("out", [BPC * NN], F32, kind="ExternalOutput").ap()

    with tile.TileContext(nc) as tc:
        with (
            tc.tile_pool(name="wp", bufs=1) as wp,
            tc.tile_pool(name="xp", bufs=3) as xp,
            tc.tile_pool(name="fp", bufs=2) as fp,
            tc.tile_pool(name="hp", bufs=2) as hp,
            tc.tile_pool(name="ap", bufs=1) as ac,
            tc.tile_pool(name="ps1", bufs=4, space="PSUM") as ps1,
            tc.tile_pool(name="ps2", bufs=2, space="PSUM") as ps2,
            tc.tile_pool(name="ps3", bufs=2, space="PSUM") as ps3,
        ):
            # ---- biases / small tensors on the HWDGE (sync) ring ----
            bso_sb = wp.tile([128, OC1], F32)
            nc.sync.dma_start(bso_sb[:], bso_d[:])
            bc_sb = wp.tile([128, OC1], F32)
            nc.sync.dma_start(bc_sb[:], bc_d[:])
            b1_sb = wp.tile([128, OC2], F32)
            nc.sync.dma_start(b1_sb[:], b1_d[:])
            w2_sb = wp.tile([128, OC2], F16)
            nc.sync.dma_start(w2_sb[:], w2_d[:])
            # big weights ride the ordered gpsimd chain (see _body): wso
            # first, wc/w1 at their consumption positions. p-major DRAM
            # layout -> one contiguous 16KB descriptor per partition.
            wso_sb = wp.tile([128, KC1 * 512], F16)
            wc_sb = wp.tile([128, KC1 * 512], F16)
            w1_sb = wp.tile([128, KC2 * 256], F16)

            out_sb = ac.tile([1, BPC * NN], F32)
            bits_sb = ac.tile([1, BPC * NN], F32)

            import contextlib
            loop_cm = tc.For_i(0, loop, 1) if loop > 1 else contextlib.nullcontext()
            with loop_cm:
                _body(nc, tc, repeat, xso_d, xc_d, out_d,
                      wso_sb, wc_sb, w1_sb, w2_sb, bso_sb, bc_sb, b1_sb,
                      out_sb, bits_sb, xp, fp, hp, ps1, ps2, ps3, thresh,
                      weight_dram=(wso_d, wc_d, w1_d))

    nc.compile()
    return nc


def _body(nc, tc, repeat, xso_d, xc_d, out_d,
          wso_sb, wc_sb, w1_sb, w2_sb, bso_sb, bc_sb, b1_sb,
          out_sb, bits_sb, xp, fp, hp, ps1, ps2, ps3, thresh,
          weight_dram=None):
    from concourse.tile import add_dep_helper

    # All big HBM reads ride one ordered gpsimd stream, chained with
    # stride 4 (transfer i waits on i-4's completion): the SDMA engines
    # round-robin across everything outstanding, so without this the
    # first-needed transfer finishes no earlier than the whole burst;
    # with it the stream drains in consumption order with two transfers
    # of lookahead.
    chain = []

    def chained_dma(out_ap, in_ap):
        h = nc.gpsimd.dma_start(out_ap, in_ap)
        if len(chain) >= 4:
            add_dep_helper(h.ins, chain[-4].ins, reason="x-stream order")
        chain.append(h)

    wso_d, wc_d, w1_d = weight_dram
    chained_dma(wso_sb[:], wso_d[:])

    if True:
        if True:
            for _rep in range(repeat):
                for b in range(BPC):
                    # ---- load x (fp32 HBM -> fp16 SBUF, cast in DMA) ----
                    # partition p holds channels 16p..16p+15: per-partition
                    # contiguous 16*588 fp32 run in DRAM.
                    # split into sub-DMAs of 4 k-chunks so layer-1 matmuls
                    # start as soon as the first chunks land instead of
                    # waiting for the whole 4.8MB transfer
                    SUB = 4
                    JS = KC1 // SUB
                    xso_sb = xp.tile([128, KC1 * NHW], F16, tag="xso")
                    xso_v = xso_d[b].rearrange("(p j) m -> p j m", p=128)
                    xso_t = xso_sb.rearrange("p (j m) -> p j m", j=KC1)
                    for s in range(SUB):
                        chained_dma(
                            xso_t[:, JS * s:JS * (s + 1), :],
                            xso_v[:, JS * s:JS * (s + 1), :])
                    xc_sb = xp.tile([128, KC1 * NHW], F16, tag="xc")
                    xc_v = xc_d[b].rearrange("(p j) m -> p j m", p=128)
                    xc_t = xc_sb.rearrange("p (j m) -> p j m", j=KC1)
                    for s in range(SUB):
                        chained_dma(
                            xc_t[:, JS * s:JS * (s + 1), :],
                            xc_v[:, JS * s:JS * (s + 1), :])
                    if _rep == 0 and b == 0:
                        # wc needed when branch-c compute starts (~1 branch
                        # of PE work away); w1 a little after that
                        chained_dma(wc_sb[:], wc_d[:])
                        chained_dma(w1_sb[:], w1_d[:])

                    # ---- layer 1: f = lrelu(A @ x + a), fp16 out ----
                    f_sb = fp.tile([128, 2 * OC1 * NHW], F16, tag="f")
                    for br, (x_sb, w_sb, bias_sb) in enumerate(
                        ((xso_sb, wso_sb, bso_sb), (xc_sb, wc_sb, bc_sb))
                    ):
                        for m in range(2):
                            for o in range(OC1):
                                ps = ps1.tile([128, MT], F32, tag="ps1")
                                for k in range(KC1):
                                    nc.tensor.matmul(
                                        ps[:],
                                        lhsT=w_sb[:, k * 512 + o * 128:
                                                  k * 512 + o * 128 + 128],
                                        rhs=x_sb[:, k * NHW + m * MT:
                                                 k * NHW + m * MT + MT],
                                        start=(k == 0), stop=(k == KC1 - 1))
                                col = (br * OC1 + o) * NHW + m * MT
                                nc.scalar.activation(
                                    f_sb[:, col:col + MT], ps[:],
                                    mybir.ActivationFunctionType.Prelu,
                                    bias=bias_sb[:, o:o + 1], scale=1.0,
                                    alpha=SLOPE)

                    # ---- layer 2: h = lrelu(A1 @ f + a1), fp16 out ----
                    h_sb = hp.tile([128, OC2 * NHW], F16, tag="h")
                    for m in range(2):
                        for o in range(OC2):
                            ps = ps2.tile([128, MT], F32, tag="ps2")
                            for k in range(KC2):
                                nc.tensor.matmul(
                                    ps[:],
                                    lhsT=w1_sb[:, k * 256 + o * 128:
                                               k * 256 + o * 128 + 128],
                                    rhs=f_sb[:, k * NHW + m * MT:
                                             k * NHW + m * MT + MT],
                                    start=(k == 0), stop=(k == KC2 - 1))
                            col = o * NHW + m * MT
                            nc.scalar.activation(
                                h_sb[:, col:col + MT], ps[:],
                                mybir.ActivationFunctionType.Prelu,
                                bias=b1_sb[:, o:o + 1], scale=1.0, alpha=SLOPE)

                    # ---- layer 3 + mean-reduce: y = W2 @ h ; sum 49-groups ----
                    for m in range(2):
                        ps = ps3.tile([1, MT], F32, tag="ps3")
                        for q in range(OC2):
                            nc.tensor.matmul(
                                ps[:],
                                lhsT=w2_sb[:, q:q + 1],
                                rhs=h_sb[:, q * NHW + m * MT:
                                         q * NHW + m * MT + MT],
                                start=(q == 0), stop=(q == OC2 - 1))
                        off = b * NN + m * (MT // HW)
                        nc.vector.reduce_sum(
                            out_sb[0:1, off:off + MT // HW],
                            ps.rearrange("p (g x) -> p g x", x=HW),
                            axis=mybir.AxisListType.X)

                # ---- threshold: sigmoid(mean) > 0.5  <=>  sum > -49*b2 ----
                nc.vector.tensor_scalar(
                    bits_sb[:], out_sb[:], float(thresh), None,
                    mybir.AluOpType.is_gt)
                nc.sync.dma_start(out_d[:], bits_sb[0:1, :])

    nc.compile()
    return nc


_CACHE = {}


def _get_nc(thresh, repeat=1, loop=1):
    key = (round(thresh, 9), repeat, loop)
    if key not in _CACHE:
        _CACHE[key] = build_bass(thresh, repeat, loop)
    return _CACHE[key]


def kernel(**inputs):
    wso, wc, w1, w2, bso, bc, b1, thresh = _fold_params(inputs)
    xso = np.ascontiguousarray(
        np.asarray(inputs["x_so"], dtype=np.float32).reshape(B, C, NHW))
    xc = np.ascontiguousarray(
        np.asarray(inputs["x_c"], dtype=np.float32).reshape(B, C, NHW))

    nc = _get_nc(thresh)
    in_maps = []
    for i in range(N_CORES):
        in_maps.append({
            "x_so": xso[i * BPC:(i + 1) * BPC],
            "x_c": xc[i * BPC:(i + 1) * BPC],
            "wso": wso, "wc": wc, "w1": w1, "w2": w2,
            "bso": bso, "bc": bc, "b1": b1,
        })
    res = run_bass_kernel_spmd(nc, in_maps, list(range(N_CORES)))
    out = np.concatenate([res.results[i]["out"].reshape(BPC, NN)
                          for i in range(N_CORES)], axis=0)
    return np.ascontiguousarray(out.reshape(B, NN, 1).astype(np.float32))

